# revision 6
# baseline (speedup 1.0000x reference)
"""Causal single-head attention for B=8, T=2048, D=1024, HS=64 on 8 TRN2 cores.

Data-parallel over batch: core i computes batch element i entirely locally.

Per-core pipeline (fp16 compute, fp32 accumulate):
  1. cast-DMA x -> SBUF fp16 tiles; PE-transpose to xT (d on partitions)
  2. qT/kT = W.T @ xT  [64, 2048]; v natural [2048, 64] + ones col -> v_aug
  3. per k-chunk: S^T[k, q] = kT-slice.T @ qT (PSUM fp32), additive causal
     mask on the diagonal block, P^T = exp(scale * S^T) on ACT (-> fp16),
     out^T[65, q] += v_aug.T @ P^T  (row 64 = softmax denominator)
  4. transpose out^T back, divide by denominator, DMA out (fp32)

No max-subtraction in softmax: scale = 1/sqrt(2048) keeps |scale*S| < ~2.

This walrus build supports at most ONE sync wait / sync update per
instruction; Tile emits more, so we hoist extras onto InstNoOp neighbours
(see _patch_tile_for_single_wait_walrus).
"""

import math
import os

import numpy as np

import concourse.bass as bass
import concourse.mybir as mybir
import concourse.tile as tile
from concourse.bass_utils import run_bass_kernel_spmd
from concourse.vector_clock import ScopedClock
from contextlib import ExitStack

F32 = mybir.dt.float32
F16 = mybir.dt.float16

B, T, D, HS = 8, 2048, 1024, 64
NT = T // 128  # 16 row tiles
NC = D // 128  # 8 contraction chunks
SCALE = 1.0 / math.sqrt(2048.0)
NEG = -1.0e9

_patched = False


def _patch_tile_for_single_wait_walrus():
    """Split multi-wait / multi-update instructions into single-sync ones."""
    global _patched
    if _patched:
        return
    _patched = True

    orig_add = tile.TileContext._add_instruction

    def patched_add(self, inst):
        si = getattr(inst, "sync_info", None)
        if si is not None and (len(si.on_wait) > 1 or len(si.on_update) > 1):
            waits = list(si.on_wait)
            updates = list(si.on_update)
            for w in waits[:-1]:
                nop = mybir.InstNoOp(
                    name=self.nc.get_next_instruction_name(),
                    engine=inst.engine,
                    sync_info=mybir.SyncInfo(on_wait=[w], on_update=[]),
                    bass_nofuse=True,
                )
                orig_add(self, nop)
            inst.sync_info = mybir.SyncInfo(on_wait=waits[-1:], on_update=updates[:1])
            orig_add(self, inst)
            for u in updates[1:]:
                nop = mybir.InstNoOp(
                    name=self.nc.get_next_instruction_name(),
                    engine=inst.engine,
                    sync_info=mybir.SyncInfo(on_wait=[], on_update=[u]),
                    bass_nofuse=True,
                )
                orig_add(self, nop)
            return
        orig_add(self, inst)

    tile.TileContext._add_instruction = patched_add

    def patched_drain(self, tick_clock, wait_clock):
        probe = self.nc.sync.nop()
        wait_clock.add_sem_waits(
            probe.ins, ScopedClock({None: tick_clock.global_clock})
        )
        si = probe.ins.sync_info
        waits = list(si.on_wait) if si is not None else []
        if si is not None:
            probe.ins.sync_info = mybir.SyncInfo(
                on_wait=[], on_update=list(si.on_update)
            )
        for w in waits:
            n = self.nc.sync.nop()
            n.ins.sync_info = mybir.SyncInfo(on_wait=[w], on_update=[])
        self.nc.sync.drain()
        self.nc.all_engine_barrier()
        popped = self.nc._tile_sem_poison_stack.pop()
        assert popped is self._sem_poison
        self.nc.clear_and_free_semaphores(list(self.sems.allocated().values()))
        self.nc.all_engine_barrier()

    tile.TileContext._drain_and_barrier = patched_drain


def build():
    nc = bass.Bass("TRN2", target_bir_lowering=False, debug=False)
    x = nc.dram_tensor("x", [T, D], F32, kind="ExternalInput").ap()
    wq = nc.dram_tensor("wq", [D, HS], F32, kind="ExternalInput").ap()
    wk = nc.dram_tensor("wk", [D, HS], F32, kind="ExternalInput").ap()
    wv = nc.dram_tensor("wv", [D, HS], F32, kind="ExternalInput").ap()
    id16 = nc.dram_tensor("id16", [128, 128], F16, kind="ExternalInput").ap()
    id32 = nc.dram_tensor("id32", [65, 65], F32, kind="ExternalInput").ap()
    out = nc.dram_tensor("out", [T, HS], F32, kind="ExternalOutput").ap()

    with tile.TileContext(nc) as tc, ExitStack() as ctx:
        sb = ctx.enter_context(tc.tile_pool(name="sb", bufs=1))

        # ---- x load first (fp32 -> fp16 cast in DMA); transposes gate on it
        x16 = []
        for t in range(NT):
            xt = sb.tile([128, D], F16, tag=f"x16_{t}", name=f"x16_{t}")
            nc.gpsimd.dma_start(xt[:], x[128 * t : 128 * (t + 1), :])
            x16.append(xt)

        # ---- constant loads
        ident16 = sb.tile([128, 128], F16, tag="id16")
        nc.sync.dma_start(ident16[:], id16)
        ident32 = sb.tile([65, 65], F32, tag="id32")
        nc.sync.dma_start(ident32[:], id32)
        w16 = {}
        for name, w in (("q", wq), ("k", wk), ("v", wv)):
            w16[name] = sb.tile([128, NC * HS], F16, tag=f"w{name}", name=f"w16{name}")
            nc.gpsimd.dma_start(
                w16[name][:].rearrange("p (c h) -> p c h", c=NC),
                w.rearrange("(c p) h -> p c h", p=128),
            )

        # ---- xT via PE transpose; pack 8 chunk-blocks per PSUM bank
        xT = sb.tile([128, NC * T], F16, tag="xT")  # chunk c at [:, c*T:(c+1)*T]
        xT3 = xT[:].rearrange("p (c t) -> p c t", c=NC)
        with tc.tile_pool(name="ptr", bufs=2, space="PSUM") as ptr_pool:
            for t in range(NT):
                ptr = ptr_pool.tile([128, 1024], F16, tag="ptr")
                for c in range(NC):
                    nc.tensor.transpose(
                        ptr[:, 128 * c : 128 * (c + 1)],
                        x16[t][:, 128 * c : 128 * (c + 1)],
                        ident16[:],
                    )
                nc.vector.tensor_copy(
                    xT3[:, :, 128 * t : 128 * (t + 1)],
                    ptr[:].rearrange("p (c u) -> p c u", c=NC),
                )

        # ---- projections
        qT = sb.tile([64, T], F16, tag="qT")
        kT = sb.tile([64, T], F16, tag="kT")
        vaug = sb.tile([128, NT * 72], F16, tag="vaug")  # 64 v cols + ones col
        nc.gpsimd.memset(vaug[:], 1.0)
        vaug3 = vaug[:].rearrange("p (t w) -> p t w", t=NT)

        with tc.tile_pool(name="pqk", bufs=2, space="PSUM") as pqk_pool, tc.tile_pool(
            name="pv", bufs=2, space="PSUM"
        ) as pv_pool:
            for name, dstT in (("q", qT), ("k", kT)):
                for s in range(4):
                    pp = pqk_pool.tile([64, 512], F32, tag="pqk")
                    for c in range(NC):
                        nc.tensor.matmul(
                            pp[:],
                            w16[name][:, HS * c : HS * (c + 1)],
                            xT[:, T * c + 512 * s : T * c + 512 * (s + 1)],
                            start=(c == 0),
                            stop=(c == NC - 1),
                        )
                    nc.vector.tensor_copy(dstT[:, 512 * s : 512 * (s + 1)], pp[:])
            for g in range(2):  # two PSUM banks of 8 v-tiles each
                pv = pv_pool.tile([128, 512], F32, tag="pv")
                for ti in range(8):
                    t = 8 * g + ti
                    for c in range(NC):
                        nc.tensor.matmul(
                            pv[:, 64 * ti : 64 * (ti + 1)],
                            xT[:, T * c + 128 * t : T * c + 128 * (t + 1)],
                            w16["v"][:, HS * c : HS * (c + 1)],
                            start=(c == 0),
                            stop=(c == NC - 1),
                        )
                nc.vector.tensor_copy(
                    vaug3[:, 8 * g : 8 * (g + 1), 0:64],
                    pv[:].rearrange("p (t h) -> p t h", t=8),
                )

        # ---- attention, two q-halves of 1024; PV lags one k-chunk behind
        # S/exp so the PE never stalls on the exp -> PV dependency.
        out2 = out.rearrange("(g p) h -> p g h", p=128)  # [128, 16, 64]
        with tc.tile_pool(name="ps_s", bufs=3, space="PSUM") as s_pool, tc.tile_pool(
            name="pout", bufs=1, space="PSUM"
        ) as o_pool, tc.tile_pool(name="sb2", bufs=4) as sb2:

            def emit_s_exp(h, kc):
                """S^T then P^T = exp(scale*S^T); zero masked diag on gpsimd."""
                q0 = 1024 * h
                qlo = max(0, 128 * kc - q0)
                sps = s_pool.tile([128, 1024], F32, tag="ps_s", name=f"s_{h}_{kc}")
                segs = [(qlo, 512), (512, 1024)] if qlo < 512 else [(qlo, 1024)]
                for a, b in segs:
                    nc.tensor.matmul(
                        sps[:, a:b],
                        kT[:, 128 * kc : 128 * (kc + 1)],
                        qT[:, q0 + a : q0 + b],
                        start=True,
                        stop=True,
                    )
                pT = sb2.tile([128, 1024], F16, tag="pT", name=f"pT_{h}_{kc}")
                nc.scalar.activation(
                    pT[:, qlo:1024],
                    sps[:, qlo:1024],
                    mybir.ActivationFunctionType.Exp,
                    scale=SCALE,
                )
                if kc >= 8 * h:
                    # causal: zero P^T[k, q] where q < k inside the diag block
                    nc.gpsimd.affine_select(
                        out=pT[:, qlo : qlo + 128],
                        in_=pT[:, qlo : qlo + 128],
                        compare_op=mybir.AluOpType.is_ge,
                        fill=0.0,
                        base=0,
                        pattern=[[1, 128]],
                        channel_multiplier=-1,
                    )
                return qlo, pT

            for h in range(2):
                q0 = 1024 * h
                n_kc = 8 * h + 8
                oT = o_pool.tile([65, 1024], F32, tag="pout", name=f"oT_{h}")

                def emit_pv(h, kc, qlo, pT, n_kc, oT):
                    for bk in range(2):
                        a = max(qlo, 512 * bk)
                        b = 512 * (bk + 1)
                        if a >= b:
                            continue
                        nc.tensor.matmul(
                            oT[:, a:b],
                            vaug3[:, kc, 0:65],
                            pT[:, a:b],
                            start=(kc == 0),
                            stop=(kc == n_kc - 1),
                        )

                prev = None
                for kc in range(n_kc):
                    cur = emit_s_exp(h, kc)
                    if prev is not None:
                        emit_pv(h, kc - 1, *prev, n_kc, oT)
                    prev = cur
                emit_pv(h, n_kc - 1, *prev, n_kc, oT)

                # normalize + emit this half (out-transposes share ps_s slots)
                oT_sb = sb2.tile([65, 1024], F32, tag="oT_sb", name=f"oTsb_{h}")
                nc.scalar.copy(oT_sb[:], oT[:])
                r32 = sb2.tile([128, 8], F32, tag="r32", name=f"r32_{h}")
                out_sb = sb2.tile([128, 512], F32, tag="out_sb", name=f"osb_{h}")
                for qb in range(8):
                    j = qb % 4
                    if j == 0:
                        otr = s_pool.tile(
                            [128, 512], F32, tag="ps_s", name=f"otr_{h}_{qb}"
                        )
                    nc.tensor.transpose(
                        otr[:, 128 * j : 128 * j + 65],
                        oT_sb[:, 128 * qb : 128 * (qb + 1)],
                        ident32[:],
                    )
                    nc.vector.reciprocal(
                        r32[:, qb : qb + 1], otr[:, 128 * j + 64 : 128 * j + 65]
                    )
                    nc.vector.tensor_scalar_mul(
                        out_sb[:, 64 * qb : 64 * (qb + 1)],
                        otr[:, 128 * j : 128 * j + 64],
                        r32[:, qb : qb + 1],
                    )
                nc.sync.dma_start(
                    out2[:, 8 * h : 8 * (h + 1), :],
                    out_sb[:].rearrange("p (g w) -> p g w", g=8),
                )

    return nc


_nc_cache = None


def _get_nc():
    global _nc_cache
    if _nc_cache is None:
        _patch_tile_for_single_wait_walrus()
        _nc_cache = build()
    return _nc_cache


def _make_in_maps(x, Wq, Wk, Wv):
    id16 = np.eye(128, dtype=np.float16)
    id32 = np.eye(65, dtype=np.float32)
    x = np.ascontiguousarray(np.asarray(x, dtype=np.float32))
    Wq = np.ascontiguousarray(np.asarray(Wq, dtype=np.float32))
    Wk = np.ascontiguousarray(np.asarray(Wk, dtype=np.float32))
    Wv = np.ascontiguousarray(np.asarray(Wv, dtype=np.float32))
    return [
        {
            "x": x[i],
            "wq": Wq,
            "wk": Wk,
            "wv": Wv,
            "id16": id16,
            "id32": id32,
        }
        for i in range(B)
    ]


def run(x, Wq, Wk, Wv, trace=False):
    nc = _get_nc()
    in_maps = _make_in_maps(x, Wq, Wk, Wv)
    res = run_bass_kernel_spmd(nc, in_maps, core_ids=list(range(B)), trace=trace)
    out = np.stack([res.results[i]["out"] for i in range(B)]).astype(np.float32)
    return out, res


def kernel(x, Wq, Wk, Wv):
    out, _ = run(x, Wq, Wk, Wv, trace=bool(os.environ.get("KERNEL_TRACE")))
    return out


# revision 8
# speedup vs baseline: 1.0055x; 1.0055x over previous
"""Causal single-head attention for B=8, T=2048, D=1024, HS=64 on 8 TRN2 cores.

Data-parallel over batch: core i computes batch element i entirely locally.

Per-core pipeline (fp16 compute, fp32 accumulate):
  1. cast-DMA x -> SBUF fp16 tiles; PE-transpose to xT (d on partitions)
  2. qT/kT = W.T @ xT  [64, 2048]; v natural [2048, 64] + ones col -> v_aug
  3. per k-chunk: S^T[k, q] = kT-slice.T @ qT (PSUM fp32), additive causal
     mask on the diagonal block, P^T = exp(scale * S^T) on ACT (-> fp16),
     out^T[65, q] += v_aug.T @ P^T  (row 64 = softmax denominator)
  4. transpose out^T back, divide by denominator, DMA out (fp32)

No max-subtraction in softmax: scale = 1/sqrt(2048) keeps |scale*S| < ~2.

This walrus build supports at most ONE sync wait / sync update per
instruction; Tile emits more, so we hoist extras onto InstNoOp neighbours
(see _patch_tile_for_single_wait_walrus).
"""

import math
import os

import numpy as np

import concourse.bass as bass
import concourse.mybir as mybir
import concourse.tile as tile
from concourse.bass_utils import run_bass_kernel_spmd
from concourse.vector_clock import ScopedClock
from contextlib import ExitStack

F32 = mybir.dt.float32
F16 = mybir.dt.float16

B, T, D, HS = 8, 2048, 1024, 64
NT = T // 128  # 16 row tiles
NC = D // 128  # 8 contraction chunks
SCALE = 1.0 / math.sqrt(2048.0)
NEG = -1.0e9

_patched = False


def _patch_tile_for_single_wait_walrus():
    """Split multi-wait / multi-update instructions into single-sync ones."""
    global _patched
    if _patched:
        return
    _patched = True

    orig_add = tile.TileContext._add_instruction

    def patched_add(self, inst):
        si = getattr(inst, "sync_info", None)
        if si is not None and (len(si.on_wait) > 1 or len(si.on_update) > 1):
            waits = list(si.on_wait)
            updates = list(si.on_update)
            for w in waits[:-1]:
                nop = mybir.InstNoOp(
                    name=self.nc.get_next_instruction_name(),
                    engine=inst.engine,
                    sync_info=mybir.SyncInfo(on_wait=[w], on_update=[]),
                    bass_nofuse=True,
                )
                orig_add(self, nop)
            inst.sync_info = mybir.SyncInfo(on_wait=waits[-1:], on_update=updates[:1])
            orig_add(self, inst)
            for u in updates[1:]:
                nop = mybir.InstNoOp(
                    name=self.nc.get_next_instruction_name(),
                    engine=inst.engine,
                    sync_info=mybir.SyncInfo(on_wait=[], on_update=[u]),
                    bass_nofuse=True,
                )
                orig_add(self, nop)
            return
        orig_add(self, inst)

    tile.TileContext._add_instruction = patched_add

    def patched_drain(self, tick_clock, wait_clock):
        probe = self.nc.sync.nop()
        wait_clock.add_sem_waits(
            probe.ins, ScopedClock({None: tick_clock.global_clock})
        )
        si = probe.ins.sync_info
        waits = list(si.on_wait) if si is not None else []
        if si is not None:
            probe.ins.sync_info = mybir.SyncInfo(
                on_wait=[], on_update=list(si.on_update)
            )
        for w in waits:
            n = self.nc.sync.nop()
            n.ins.sync_info = mybir.SyncInfo(on_wait=[w], on_update=[])
        self.nc.sync.drain()
        self.nc.all_engine_barrier()
        popped = self.nc._tile_sem_poison_stack.pop()
        assert popped is self._sem_poison
        self.nc.clear_and_free_semaphores(list(self.sems.allocated().values()))
        self.nc.all_engine_barrier()

    tile.TileContext._drain_and_barrier = patched_drain


def build():
    nc = bass.Bass("TRN2", target_bir_lowering=False, debug=False)
    x = nc.dram_tensor("x", [T, D], F32, kind="ExternalInput").ap()
    wq = nc.dram_tensor("wq", [D, HS], F32, kind="ExternalInput").ap()
    wk = nc.dram_tensor("wk", [D, HS], F32, kind="ExternalInput").ap()
    wv = nc.dram_tensor("wv", [D, HS], F32, kind="ExternalInput").ap()
    id16 = nc.dram_tensor("id16", [128, 128], F16, kind="ExternalInput").ap()
    id32 = nc.dram_tensor("id32", [65, 65], F32, kind="ExternalInput").ap()
    out = nc.dram_tensor("out", [T, HS], F32, kind="ExternalOutput").ap()

    with tile.TileContext(nc) as tc, ExitStack() as ctx:
        sb = ctx.enter_context(tc.tile_pool(name="sb", bufs=1))

        # ---- constant loads first (tiny; W gates the projections)
        ident16 = sb.tile([128, 128], F16, tag="id16")
        nc.sync.dma_start(ident16[:], id16)
        ident32 = sb.tile([65, 65], F32, tag="id32")
        nc.sync.dma_start(ident32[:], id32)
        w16 = {}
        for name, w in (("q", wq), ("k", wk), ("v", wv)):
            w16[name] = sb.tile([128, NC * HS], F16, tag=f"w{name}", name=f"w16{name}")
            nc.gpsimd.dma_start(
                w16[name][:].rearrange("p (c h) -> p c h", c=NC),
                w.rearrange("(c p) h -> p c h", p=128),
            )

        # ---- x load (fp32 -> fp16 cast in DMA). Chain the DMAs (3 in
        # flight) so early tiles complete early instead of all transfers
        # round-robining to completion together.
        x16 = []
        x_dmas = []
        for t in range(NT):
            xt = sb.tile([128, D], F16, tag=f"x16_{t}", name=f"x16_{t}")
            dma = nc.gpsimd.dma_start(xt[:], x[128 * t : 128 * (t + 1), :])
            if t >= 3:
                bass._add_dep_helper(
                    dma.ins, x_dmas[t - 3].ins, sync=True, reason="dma throttle"
                )
            x_dmas.append(dma)
            x16.append(xt)

        # ---- xT via PE transpose; pack 8 chunk-blocks per PSUM bank
        xT = sb.tile([128, NC * T], F16, tag="xT")  # chunk c at [:, c*T:(c+1)*T]
        xT3 = xT[:].rearrange("p (c t) -> p c t", c=NC)
        with tc.tile_pool(name="ptr", bufs=2, space="PSUM") as ptr_pool:
            for t in range(NT):
                ptr = ptr_pool.tile([128, 1024], F16, tag="ptr")
                for c in range(NC):
                    nc.tensor.transpose(
                        ptr[:, 128 * c : 128 * (c + 1)],
                        x16[t][:, 128 * c : 128 * (c + 1)],
                        ident16[:],
                    )
                nc.vector.tensor_copy(
                    xT3[:, :, 128 * t : 128 * (t + 1)],
                    ptr[:].rearrange("p (c u) -> p c u", c=NC),
                )

        # ---- projections
        qT = sb.tile([64, T], F16, tag="qT")
        kT = sb.tile([64, T], F16, tag="kT")
        vaug = sb.tile([128, NT * 72], F16, tag="vaug")  # 64 v cols + ones col
        nc.gpsimd.memset(vaug[:], 1.0)
        vaug3 = vaug[:].rearrange("p (t w) -> p t w", t=NT)

        with tc.tile_pool(name="pqk", bufs=2, space="PSUM") as pqk_pool, tc.tile_pool(
            name="pv", bufs=2, space="PSUM"
        ) as pv_pool:
            for name, dstT in (("q", qT), ("k", kT)):
                for s in range(4):
                    pp = pqk_pool.tile([64, 512], F32, tag="pqk")
                    for c in range(NC):
                        nc.tensor.matmul(
                            pp[:],
                            w16[name][:, HS * c : HS * (c + 1)],
                            xT[:, T * c + 512 * s : T * c + 512 * (s + 1)],
                            start=(c == 0),
                            stop=(c == NC - 1),
                        )
                    nc.vector.tensor_copy(dstT[:, 512 * s : 512 * (s + 1)], pp[:])
            for g in range(2):  # two PSUM banks of 8 v-tiles each
                pv = pv_pool.tile([128, 512], F32, tag="pv")
                for ti in range(8):
                    t = 8 * g + ti
                    for c in range(NC):
                        nc.tensor.matmul(
                            pv[:, 64 * ti : 64 * (ti + 1)],
                            xT[:, T * c + 128 * t : T * c + 128 * (t + 1)],
                            w16["v"][:, HS * c : HS * (c + 1)],
                            start=(c == 0),
                            stop=(c == NC - 1),
                        )
                nc.vector.tensor_copy(
                    vaug3[:, 8 * g : 8 * (g + 1), 0:64],
                    pv[:].rearrange("p (t h) -> p t h", t=8),
                )

        # ---- attention, two q-halves of 1024; PV lags one k-chunk behind
        # S/exp so the PE never stalls on the exp -> PV dependency.
        out2 = out.rearrange("(g p) h -> p g h", p=128)  # [128, 16, 64]
        with tc.tile_pool(name="ps_s", bufs=3, space="PSUM") as s_pool, tc.tile_pool(
            name="pout", bufs=1, space="PSUM"
        ) as o_pool, tc.tile_pool(name="sb2", bufs=4) as sb2:

            def emit_s_exp(h, kc):
                """S^T then P^T = exp(scale*S^T); zero masked diag on gpsimd."""
                q0 = 1024 * h
                qlo = max(0, 128 * kc - q0)
                sps = s_pool.tile([128, 1024], F32, tag="ps_s", name=f"s_{h}_{kc}")
                segs = [(qlo, 512), (512, 1024)] if qlo < 512 else [(qlo, 1024)]
                for a, b in segs:
                    nc.tensor.matmul(
                        sps[:, a:b],
                        kT[:, 128 * kc : 128 * (kc + 1)],
                        qT[:, q0 + a : q0 + b],
                        start=True,
                        stop=True,
                    )
                pT = sb2.tile([128, 1024], F16, tag="pT", name=f"pT_{h}_{kc}")
                nc.scalar.activation(
                    pT[:, qlo:1024],
                    sps[:, qlo:1024],
                    mybir.ActivationFunctionType.Exp,
                    scale=SCALE,
                )
                if kc >= 8 * h:
                    # causal: zero P^T[k, q] where q < k inside the diag block
                    nc.gpsimd.affine_select(
                        out=pT[:, qlo : qlo + 128],
                        in_=pT[:, qlo : qlo + 128],
                        compare_op=mybir.AluOpType.is_ge,
                        fill=0.0,
                        base=0,
                        pattern=[[1, 128]],
                        channel_multiplier=-1,
                    )
                return qlo, pT

            for h in range(2):
                q0 = 1024 * h
                n_kc = 8 * h + 8
                oT = o_pool.tile([65, 1024], F32, tag="pout", name=f"oT_{h}")

                def emit_pv(h, kc, qlo, pT, n_kc, oT):
                    for bk in range(2):
                        a = max(qlo, 512 * bk)
                        b = 512 * (bk + 1)
                        if a >= b:
                            continue
                        nc.tensor.matmul(
                            oT[:, a:b],
                            vaug3[:, kc, 0:65],
                            pT[:, a:b],
                            start=(kc == 0),
                            stop=(kc == n_kc - 1),
                        )

                pending = []
                for kc in range(n_kc):
                    pending.append((kc, emit_s_exp(h, kc)))
                    if len(pending) > 2:  # PV two k-chunks behind S/exp
                        pkc, pcur = pending.pop(0)
                        emit_pv(h, pkc, *pcur, n_kc, oT)
                for pkc, pcur in pending:
                    emit_pv(h, pkc, *pcur, n_kc, oT)

                # normalize + emit this half (out-transposes share ps_s slots)
                oT_sb = sb2.tile([65, 1024], F32, tag="oT_sb", name=f"oTsb_{h}")
                nc.scalar.copy(oT_sb[:], oT[:])
                r32 = sb2.tile([128, 8], F32, tag="r32", name=f"r32_{h}")
                out_sb = sb2.tile([128, 512], F32, tag="out_sb", name=f"osb_{h}")
                for qb in range(8):
                    j = qb % 4
                    if j == 0:
                        otr = s_pool.tile(
                            [128, 512], F32, tag="ps_s", name=f"otr_{h}_{qb}"
                        )
                    nc.tensor.transpose(
                        otr[:, 128 * j : 128 * j + 65],
                        oT_sb[:, 128 * qb : 128 * (qb + 1)],
                        ident32[:],
                    )
                    nc.vector.reciprocal(
                        r32[:, qb : qb + 1], otr[:, 128 * j + 64 : 128 * j + 65]
                    )
                    nc.vector.tensor_scalar_mul(
                        out_sb[:, 64 * qb : 64 * (qb + 1)],
                        otr[:, 128 * j : 128 * j + 64],
                        r32[:, qb : qb + 1],
                    )
                nc.sync.dma_start(
                    out2[:, 8 * h : 8 * (h + 1), :],
                    out_sb[:].rearrange("p (g w) -> p g w", g=8),
                )

    return nc


_nc_cache = None


def _get_nc():
    global _nc_cache
    if _nc_cache is None:
        _patch_tile_for_single_wait_walrus()
        _nc_cache = build()
    return _nc_cache


def _make_in_maps(x, Wq, Wk, Wv):
    id16 = np.eye(128, dtype=np.float16)
    id32 = np.eye(65, dtype=np.float32)
    x = np.ascontiguousarray(np.asarray(x, dtype=np.float32))
    Wq = np.ascontiguousarray(np.asarray(Wq, dtype=np.float32))
    Wk = np.ascontiguousarray(np.asarray(Wk, dtype=np.float32))
    Wv = np.ascontiguousarray(np.asarray(Wv, dtype=np.float32))
    return [
        {
            "x": x[i],
            "wq": Wq,
            "wk": Wk,
            "wv": Wv,
            "id16": id16,
            "id32": id32,
        }
        for i in range(B)
    ]


def run(x, Wq, Wk, Wv, trace=False):
    nc = _get_nc()
    in_maps = _make_in_maps(x, Wq, Wk, Wv)
    res = run_bass_kernel_spmd(nc, in_maps, core_ids=list(range(B)), trace=trace)
    out = np.stack([res.results[i]["out"] for i in range(B)]).astype(np.float32)
    return out, res


def kernel(x, Wq, Wk, Wv):
    out, _ = run(x, Wq, Wk, Wv, trace=bool(os.environ.get("KERNEL_TRACE")))
    return out


# revision 9
# speedup vs baseline: 1.2551x; 1.2483x over previous
"""Causal single-head attention for B=8, T=2048, D=1024, HS=64 on 8 TRN2 cores.

Data-parallel over batch: core i computes batch element i entirely locally.

Per-core pipeline (fp16 compute, fp32 accumulate):
  1. cast-DMA x -> SBUF fp16 tiles; PE-transpose to xT (d on partitions)
  2. qT/kT = W.T @ xT  [64, 2048]; v natural [2048, 64] + ones col -> v_aug
  3. per k-chunk: S^T[k, q] = kT-slice.T @ qT (PSUM fp32), additive causal
     mask on the diagonal block, P^T = exp(scale * S^T) on ACT (-> fp16),
     out^T[65, q] += v_aug.T @ P^T  (row 64 = softmax denominator)
  4. transpose out^T back, divide by denominator, DMA out (fp32)

No max-subtraction in softmax: scale = 1/sqrt(2048) keeps |scale*S| < ~2.

This walrus build supports at most ONE sync wait / sync update per
instruction; Tile emits more, so we hoist extras onto InstNoOp neighbours
(see _patch_tile_for_single_wait_walrus).
"""

import math
import os

import numpy as np

import concourse.bass as bass
import concourse.mybir as mybir
import concourse.tile as tile
from concourse.bass_utils import run_bass_kernel_spmd
from concourse.vector_clock import ScopedClock
from contextlib import ExitStack

F32 = mybir.dt.float32
F16 = mybir.dt.float16

B, T, D, HS = 8, 2048, 1024, 64
NT = T // 128  # 16 row tiles
NC = D // 128  # 8 contraction chunks
SCALE = 1.0 / math.sqrt(2048.0)
NEG = -1.0e9

_patched = False


def _patch_tile_for_single_wait_walrus():
    """Split multi-wait / multi-update instructions into single-sync ones."""
    global _patched
    if _patched:
        return
    _patched = True

    orig_add = tile.TileContext._add_instruction

    def patched_add(self, inst):
        si = getattr(inst, "sync_info", None)
        if si is not None and (len(si.on_wait) > 1 or len(si.on_update) > 1):
            waits = list(si.on_wait)
            updates = list(si.on_update)
            for w in waits[:-1]:
                nop = mybir.InstNoOp(
                    name=self.nc.get_next_instruction_name(),
                    engine=inst.engine,
                    sync_info=mybir.SyncInfo(on_wait=[w], on_update=[]),
                    bass_nofuse=True,
                )
                orig_add(self, nop)
            inst.sync_info = mybir.SyncInfo(on_wait=waits[-1:], on_update=updates[:1])
            orig_add(self, inst)
            for u in updates[1:]:
                nop = mybir.InstNoOp(
                    name=self.nc.get_next_instruction_name(),
                    engine=inst.engine,
                    sync_info=mybir.SyncInfo(on_wait=[], on_update=[u]),
                    bass_nofuse=True,
                )
                orig_add(self, nop)
            return
        orig_add(self, inst)

    tile.TileContext._add_instruction = patched_add

    def patched_drain(self, tick_clock, wait_clock):
        probe = self.nc.sync.nop()
        wait_clock.add_sem_waits(
            probe.ins, ScopedClock({None: tick_clock.global_clock})
        )
        si = probe.ins.sync_info
        waits = list(si.on_wait) if si is not None else []
        if si is not None:
            probe.ins.sync_info = mybir.SyncInfo(
                on_wait=[], on_update=list(si.on_update)
            )
        for w in waits:
            n = self.nc.sync.nop()
            n.ins.sync_info = mybir.SyncInfo(on_wait=[w], on_update=[])
        self.nc.sync.drain()
        self.nc.all_engine_barrier()
        popped = self.nc._tile_sem_poison_stack.pop()
        assert popped is self._sem_poison
        self.nc.clear_and_free_semaphores(list(self.sems.allocated().values()))
        self.nc.all_engine_barrier()

    tile.TileContext._drain_and_barrier = patched_drain


def build():
    nc = bass.Bass("TRN2", target_bir_lowering=False, debug=False)
    x = nc.dram_tensor("x", [T, D], F32, kind="ExternalInput").ap()
    wq = nc.dram_tensor("wq", [D, HS], F32, kind="ExternalInput").ap()
    wk = nc.dram_tensor("wk", [D, HS], F32, kind="ExternalInput").ap()
    wv = nc.dram_tensor("wv", [D, HS], F32, kind="ExternalInput").ap()
    id16 = nc.dram_tensor("id16", [128, 128], F16, kind="ExternalInput").ap()
    id32 = nc.dram_tensor("id32", [65, 65], F32, kind="ExternalInput").ap()
    out = nc.dram_tensor("out", [T, HS], F32, kind="ExternalOutput").ap()

    with tile.TileContext(nc) as tc, ExitStack() as ctx:
        sb = ctx.enter_context(tc.tile_pool(name="sb", bufs=1))
        sb2 = ctx.enter_context(tc.tile_pool(name="sb2", bufs=4))
        # one shared PSUM pool: 3 slots x 4KB (2 banks) + oT accumulators
        wk_pool = ctx.enter_context(tc.tile_pool(name="work", bufs=3, space="PSUM"))
        o_pool = ctx.enter_context(tc.tile_pool(name="pout", bufs=1, space="PSUM"))

        def wtile(shape, dtype, name):
            return wk_pool.tile(shape, dtype, tag="work", name=name)

        # ---- constants first
        ident16 = sb.tile([128, 128], F16, tag="id16")
        nc.sync.dma_start(ident16[:], id16)
        ident32 = sb.tile([65, 65], F32, tag="id32")
        nc.sync.dma_start(ident32[:], id32)
        w16 = {}
        for name, w in (("q", wq), ("k", wk), ("v", wv)):
            w16[name] = sb.tile([128, NC * HS], F16, tag=f"w{name}", name=f"w16{name}")
            nc.gpsimd.dma_start(
                w16[name][:].rearrange("p (c h) -> p c h", c=NC),
                w.rearrange("(c p) h -> p c h", p=128),
            )
        # preload the exp table set long before the first real exp
        warm = sb.tile([1, 2], F32, tag="warm")
        nc.scalar.activation(
            warm[:], ident32[0:1, 0:2], mybir.ActivationFunctionType.Exp
        )

        # ---- x cast-loads, throttled to ~4 in flight
        x16 = []
        x_dmas = []
        for t in range(NT):
            xt = sb.tile([128, D], F16, tag=f"x16_{t}", name=f"x16_{t}")
            dma = nc.gpsimd.dma_start(xt[:], x[128 * t : 128 * (t + 1), :])
            if t >= 4:
                bass._add_dep_helper(
                    dma.ins, x_dmas[t - 4].ins, sync=True, reason="dma throttle"
                )
            x_dmas.append(dma)
            x16.append(xt)

        xT = sb.tile([128, NC * T], F16, tag="xT")
        xT3 = xT[:].rearrange("p (c t) -> p c t", c=NC)
        qT = sb.tile([64, T], F16, tag="qT")
        kT = sb.tile([64, T], F16, tag="kT")
        vaug = sb.tile([128, NT * 72], F16, tag="vaug")
        nc.gpsimd.memset(vaug[:], 1.0)
        vaug3 = vaug[:].rearrange("p (t w) -> p t w", t=NT)
        out2 = out.rearrange("(g p) h -> p g h", p=128)

        def emit_transpose_group(ts):
            for t in ts:
                ptr = wtile([128, 1024], F16, f"ptr_{t}")
                for c in range(NC):
                    nc.tensor.transpose(
                        ptr[:, 128 * c : 128 * (c + 1)],
                        x16[t][:, 128 * c : 128 * (c + 1)],
                        ident16[:],
                    )
                nc.vector.tensor_copy(
                    xT3[:, :, 128 * t : 128 * (t + 1)],
                    ptr[:].rearrange("p (c u) -> p c u", c=NC),
                )

        def emit_qk_slice(s):
            for name, dstT in (("q", qT), ("k", kT)):
                pp = wtile([64, 512], F32, f"p{name}_{s}")
                for c in range(NC):
                    nc.tensor.matmul(
                        pp[:],
                        w16[name][:, HS * c : HS * (c + 1)],
                        xT[:, T * c + 512 * s : T * c + 512 * (s + 1)],
                        start=(c == 0),
                        stop=(c == NC - 1),
                    )
                nc.vector.tensor_copy(dstT[:, 512 * s : 512 * (s + 1)], pp[:])

        def emit_v_group(g):
            pv = wtile([128, 512], F32, f"pv_{g}")
            for ti in range(8):
                t = 8 * g + ti
                for c in range(NC):
                    nc.tensor.matmul(
                        pv[:, 64 * ti : 64 * (ti + 1)],
                        xT[:, T * c + 128 * t : T * c + 128 * (t + 1)],
                        w16["v"][:, HS * c : HS * (c + 1)],
                        start=(c == 0),
                        stop=(c == NC - 1),
                    )
            nc.vector.tensor_copy(
                vaug3[:, 8 * g : 8 * (g + 1), 0:64],
                pv[:].rearrange("p (t h) -> p t h", t=8),
            )

        def emit_s_exp(h, kc):
            q0 = 1024 * h
            qlo = max(0, 128 * kc - q0)
            sps = wtile([128, 1024], F32, f"s_{h}_{kc}")
            segs = [(qlo, 512), (512, 1024)] if qlo < 512 else [(qlo, 1024)]
            for a, b in segs:
                nc.tensor.matmul(
                    sps[:, a:b],
                    kT[:, 128 * kc : 128 * (kc + 1)],
                    qT[:, q0 + a : q0 + b],
                    start=True,
                    stop=True,
                )
            pT = sb2.tile([128, 1024], F16, tag="pT", name=f"pT_{h}_{kc}")
            nc.scalar.activation(
                pT[:, qlo:1024],
                sps[:, qlo:1024],
                mybir.ActivationFunctionType.Exp,
                scale=SCALE,
            )
            if kc >= 8 * h:
                nc.gpsimd.affine_select(
                    out=pT[:, qlo : qlo + 128],
                    in_=pT[:, qlo : qlo + 128],
                    compare_op=mybir.AluOpType.is_ge,
                    fill=0.0,
                    base=0,
                    pattern=[[1, 128]],
                    channel_multiplier=-1,
                )
            return qlo, pT

        def emit_attention(h, oT):
            n_kc = 8 * h + 8

            def emit_pv(kc, qlo, pT):
                for bk in range(2):
                    a = max(qlo, 512 * bk)
                    b = 512 * (bk + 1)
                    if a >= b:
                        continue
                    nc.tensor.matmul(
                        oT[:, a:b],
                        vaug3[:, kc, 0:65],
                        pT[:, a:b],
                        start=(kc == 0),
                        stop=(kc == n_kc - 1),
                    )

            pending = []
            for kc in range(n_kc):
                pending.append((kc, emit_s_exp(h, kc)))
                if len(pending) > 2:
                    pkc, (pqlo, ppT) = pending.pop(0)
                    emit_pv(pkc, pqlo, ppT)
            for pkc, (pqlo, ppT) in pending:
                emit_pv(pkc, pqlo, ppT)

        def emit_tail(h, oT):
            oT_sb = sb2.tile([65, 1024], F32, tag="oT_sb", name=f"oTsb_{h}")
            nc.vector.tensor_copy(oT_sb[:], oT[:])
            r32 = sb2.tile([128, 8], F32, tag="r32", name=f"r32_{h}")
            out_sb = sb2.tile([128, 512], F32, tag="out_sb", name=f"osb_{h}")
            for qb in range(8):
                j = qb % 4
                if j == 0:
                    otr = wtile([128, 512], F32, f"otr_{h}_{qb}")
                nc.tensor.transpose(
                    otr[:, 128 * j : 128 * j + 65],
                    oT_sb[:, 128 * qb : 128 * (qb + 1)],
                    ident32[:],
                )
                nc.vector.reciprocal(
                    r32[:, qb : qb + 1], otr[:, 128 * j + 64 : 128 * j + 65]
                )
                nc.vector.tensor_scalar_mul(
                    out_sb[:, 64 * qb : 64 * (qb + 1)],
                    otr[:, 128 * j : 128 * j + 64],
                    r32[:, qb : qb + 1],
                )
            nc.sync.dma_start(
                out2[:, 8 * h : 8 * (h + 1), :],
                out_sb[:].rearrange("p (g w) -> p g w", g=8),
            )

        # ---- interleaved schedule: h0 attention as soon as its inputs exist
        emit_transpose_group(range(0, 4))
        emit_qk_slice(0)
        emit_transpose_group(range(4, 8))
        emit_qk_slice(1)
        emit_v_group(0)
        oT0 = o_pool.tile([65, 1024], F32, tag="pout", name="oT_0")
        emit_attention(0, oT0)
        emit_tail(0, oT0)
        emit_transpose_group(range(8, 12))
        emit_qk_slice(2)
        emit_transpose_group(range(12, 16))
        emit_qk_slice(3)
        emit_v_group(1)
        oT1 = o_pool.tile([65, 1024], F32, tag="pout", name="oT_1")
        emit_attention(1, oT1)
        emit_tail(1, oT1)

    return nc


_nc_cache = None


def _get_nc():
    global _nc_cache
    if _nc_cache is None:
        _patch_tile_for_single_wait_walrus()
        _nc_cache = build()
    return _nc_cache


def _make_in_maps(x, Wq, Wk, Wv):
    id16 = np.eye(128, dtype=np.float16)
    id32 = np.eye(65, dtype=np.float32)
    x = np.ascontiguousarray(np.asarray(x, dtype=np.float32))
    Wq = np.ascontiguousarray(np.asarray(Wq, dtype=np.float32))
    Wk = np.ascontiguousarray(np.asarray(Wk, dtype=np.float32))
    Wv = np.ascontiguousarray(np.asarray(Wv, dtype=np.float32))
    return [
        {
            "x": x[i],
            "wq": Wq,
            "wk": Wk,
            "wv": Wv,
            "id16": id16,
            "id32": id32,
        }
        for i in range(B)
    ]


def run(x, Wq, Wk, Wv, trace=False):
    nc = _get_nc()
    in_maps = _make_in_maps(x, Wq, Wk, Wv)
    res = run_bass_kernel_spmd(nc, in_maps, core_ids=list(range(B)), trace=trace)
    out = np.stack([res.results[i]["out"] for i in range(B)]).astype(np.float32)
    return out, res


def kernel(x, Wq, Wk, Wv):
    out, _ = run(x, Wq, Wk, Wv, trace=bool(os.environ.get("KERNEL_TRACE")))
    return out
